# revision 21
# baseline (speedup 1.0000x reference)
"""Trainium2 Bass kernel for nn_AGCNBlock (gnn_message_passing).

Data-parallel over batch: 16 batches -> 8 cores x 2 batches.

Per-batch math (N=2048 nodes, D=HD=128, K=512):
  h   = (adj @ relu(adj@X) @ W1 + b1) @ ... 2-layer GCN -> h [N,HD]
  out = mean-pool(h);  att = softmax(h @ w_a);  Z = att*h
  top-512 nodes by att (desc, ties by index asc — matches jax.lax.top_k)
  A   = row-gathered adj, column-normalized -> new_adj = A adj A^T, H_out = A Z

Layout trick: only adj^T ("T", [col, row]) is kept in SBUF.  Every matmul
(including A-products) consumes T either as stationary or moving, and the
assign matrix is built directly in A^T layout by a free-axis gather of T's
columns (gpsimd ap_gather).  All matmuls run as float32r (full fp32 precision
at 1 cycle/row for moving-free >= 256) because the top-k ordering is
sensitive to ~1e-6 logit noise.
"""

import sys

sys.path.insert(0, "/opt/trn_rl_repo")

import numpy as np

import concourse.bass as bass
import concourse.bacc as bacc
import concourse.mybir as mybir
import concourse.tile as tile
from concourse.bass_utils import run_bass_kernel_spmd

F32 = mybir.dt.float32
F32R = mybir.dt.float32r
I16 = mybir.dt.int16
U32 = mybir.dt.uint32
U8 = mybir.dt.uint8
AF = mybir.ActivationFunctionType
OP = mybir.AluOpType
AX = mybir.AxisListType

import os as _os
STAGE = int(_os.environ.get("KSTAGE", "5"))

B, N, D, HD = 16, 2048, 128, 128
K = 512
NCORES = 8
BL = B // NCORES  # batches per core
NCH = N // 128    # 16 partition-chunks
EPS = 1e-10
NEG = 1e10
QUANT = 1.0 - 510.5 / 2047.0  # kth_largest: k_adj=510 -> out[0,1]=desc[511]


def r(ap):
    return ap.bitcast(F32R)


def f(ap):
    return ap.bitcast(F32)


def build_nc():
    nc = bacc.Bacc()

    adjT_h = nc.declare_dram_parameter("adjT", [BL, N, N], F32R, isOutput=False)
    x_h = nc.declare_dram_parameter("x", [BL, N, D], F32R, isOutput=False)
    mask_h = nc.declare_dram_parameter("mask", [BL, N], F32, isOutput=False)
    w1_h = nc.declare_dram_parameter("w1", [D, HD], F32, isOutput=False)
    w2_h = nc.declare_dram_parameter("w2", [HD, HD], F32, isOutput=False)
    b1_h = nc.declare_dram_parameter("b1", [HD], F32, isOutput=False)
    b2_h = nc.declare_dram_parameter("b2", [HD], F32, isOutput=False)
    wa_h = nc.declare_dram_parameter("wa", [HD], F32, isOutput=False)
    iota512_h = nc.declare_dram_parameter("iota512", [K], F32, isOutput=False)
    iota128_h = nc.declare_dram_parameter("iota128", [128], F32, isOutput=False)
    iotam_h = nc.declare_dram_parameter("iotam", [N], F32, isOutput=False)
    negidx_h = nc.declare_dram_parameter("negidx", [N], F32, isOutput=False)
    ident_h = nc.declare_dram_parameter("ident", [128, 128], F32, isOutput=False)

    out_h = nc.declare_dram_parameter("out", [BL, HD], F32, isOutput=True)
    hout_h = nc.declare_dram_parameter("hout", [BL, K, HD], F32, isOutput=True)
    nadj_h = nc.declare_dram_parameter("nadj", [BL, K, K], F32, isOutput=True)
    nmask_h = nc.declare_dram_parameter("nmask", [BL, K], F32, isOutput=True)

    # DRAM scratch for layout bounces (per batch: attm@0 idxm@2048 attC@4096 idxC@4608)
    scr_h = nc.dram_tensor("scr", [BL, 5120], F32)
    scrS_h = nc.dram_tensor("scrS", [BL, K], I16)

    from contextlib import ExitStack

    with tile.TileContext(nc) as tc, ExitStack() as ctx:
        build_tile(tc, nc, {**locals(), "ctx": ctx})
    nc.compile()
    return nc


def build_tile(tc, nc, h):
    adjT_h, x_h, mask_h = h["adjT_h"], h["x_h"], h["mask_h"]
    w1_h, w2_h, b1_h, b2_h, wa_h = h["w1_h"], h["w2_h"], h["b1_h"], h["b2_h"], h["wa_h"]
    iota512_h, iota128_h, iotam_h, negidx_h, ident_h = (
        h["iota512_h"], h["iota128_h"], h["iotam_h"], h["negidx_h"], h["ident_h"],
    )
    out_h, hout_h, nadj_h, nmask_h = h["out_h"], h["hout_h"], h["nadj_h"], h["nmask_h"]
    scr_h, scrS_h = h["scr_h"], h["scrS_h"]

    dma = nc.sync.dma_start
    ctx = h["ctx"]

    cpool = ctx.enter_context(tc.tile_pool(name="consts", bufs=1))
    tpool = ctx.enter_context(tc.tile_pool(name="T", bufs=NCH))
    xpool = ctx.enter_context(tc.tile_pool(name="x", bufs=2))
    hTpool = ctx.enter_context(tc.tile_pool(name="hT", bufs=1))
    hddpool = ctx.enter_context(tc.tile_pool(name="hdd", bufs=1))
    zpool = ctx.enter_context(tc.tile_pool(name="z", bufs=1))
    atpool = ctx.enter_context(tc.tile_pool(name="at", bufs=1))
    ypool = ctx.enter_context(tc.tile_pool(name="y", bufs=2))
    selpool = ctx.enter_context(tc.tile_pool(name="sel", bufs=2))   # [128,512] scratch
    perpool = ctx.enter_context(tc.tile_pool(name="per", bufs=1))   # per-batch persistents
    smpool = ctx.enter_context(tc.tile_pool(name="small", bufs=1))  # [128,<=16] tiles
    jkpool = ctx.enter_context(tc.tile_pool(name="junk", bufs=2))   # ttr elementwise dump

    # PSUM: 16KB/partition = 8 banks. One shared accumulate pool (tag "acc",
    # 5 slots: phase A uses 4, phase C uses 4+1), Y pipeline 1, small 1, row 1.
    psacc = ctx.enter_context(tc.tile_pool(name="psacc", bufs=5, space="PSUM"))
    psy = ctx.enter_context(tc.tile_pool(name="psy", bufs=1, space="PSUM"))
    pssm = ctx.enter_context(tc.tile_pool(name="pssm", bufs=2, space="PSUM"))
    psrow = pssm

    # ---- constants ----
    W1s = cpool.tile([128, 128], F32, tag="w1")
    dma(out=W1s, in_=w1_h[:, :])
    W2s = cpool.tile([128, 128], F32, tag="w2")
    dma(out=W2s, in_=w2_h[:, :])
    b1row = cpool.tile([128, 128], F32, tag="b1r")
    dma(out=b1row, in_=bass.AP(b1_h, 0, [[0, 128], [1, 128]]))
    b2row = cpool.tile([128, 128], F32, tag="b2r")
    dma(out=b2row, in_=bass.AP(b2_h, 0, [[0, 128], [1, 128]]))
    warow = cpool.tile([128, 128], F32, tag="war")
    dma(out=warow, in_=bass.AP(wa_h, 0, [[0, 128], [1, 128]]))
    i512row = cpool.tile([128, K], F32, tag="i512")
    dma(out=i512row, in_=bass.AP(iota512_h, 0, [[0, 128], [1, K]]))
    i128col = cpool.tile([128, 1], F32, tag="i128")
    dma(out=i128col, in_=bass.AP(iota128_h, 0, [[1, 128], [1, 1]]))
    iotamc = cpool.tile([128, NCH], F32, tag="iotam")
    dma(out=iotamc, in_=bass.AP(iotam_h, 0, [[1, 128], [128, NCH]]))
    negidxc = cpool.tile([128, NCH], F32, tag="negidx")
    dma(out=negidxc, in_=bass.AP(negidx_h, 0, [[1, 128], [128, NCH]]))
    identsb = cpool.tile([128, 128], F32, tag="ident")
    dma(out=identsb, in_=ident_h[:, :])

    ones_row1 = cpool.tile([1, 128], F32, tag="ones1")
    nc.vector.memset(ones_row1, 1.0)
    ones_f32 = cpool.tile([128, 1], F32, tag="onesf")
    nc.vector.memset(ones_f32, 1.0)

    uid = [0]

    def bc(src11, tag):
        # broadcast [1,1] -> [128,1] via PE: psum[i,0] = ones_row1[0,i]*src
        uid[0] += 1
        pb = pssm.tile([128, 1], F32, tag="pssm", name=f"bc{uid[0]}")
        nc.tensor.matmul(pb, ones_row1, src11)
        dst = smpool.tile([128, 1], F32, tag=tag, name=f"bct{uid[0]}")
        nc.vector.tensor_copy(dst, pb)
        return dst

    def xreduce(src, op, tag):
        # cross-partition reduce [128,1] -> [1,1] via PE transpose + DVE
        uid[0] += 1
        pt_ = pssm.tile([1, 128], F32, tag="pssm", name=f"xr{uid[0]}")
        nc.tensor.transpose(pt_, src, identsb)
        dst = smpool.tile([1, 1], F32, tag=tag, name=f"xrt{uid[0]}")
        nc.vector.tensor_reduce(dst, pt_, AX.X, op)
        return dst

    def xsum_all(src, tag):
        # sum over all elements of [128,F] -> [1,1] via ones matmul + DVE
        uid[0] += 1
        F_ = src.shape[-1]
        pr = psrow.tile([1, F_], F32, tag="pssm", name=f"xs{uid[0]}")
        nc.tensor.matmul(pr, ones_f32, src)
        dst = smpool.tile([1, 1], F32, tag=tag, name=f"xst{uid[0]}")
        nc.vector.tensor_reduce(dst, pr, AX.X, OP.add)
        return dst

    for b in range(BL):
        # ================= PHASE A: h-chain (float32r) =================
        mask_sb = smpool.tile([128, NCH], F32, tag="mask")
        dma(out=mask_sb, in_=bass.AP(mask_h, b * N, [[1, 128], [128, NCH]]))

        T = []
        for k in range(NCH):
            Tk = tpool.tile([128, N], F32R, tag="T")
            dma(out=Tk, in_=adjT_h[b, 128 * k : 128 * (k + 1), :])
            T.append(Tk)

        Xc = []
        for k in range(NCH):
            Xk = xpool.tile([128, 128], F32R, tag="x")
            dma(out=Xk, in_=x_h[b, 128 * k : 128 * (k + 1), :])
            Xc.append(Xk)

        # h1T[d, m] = sum_n X[n,d] * adj[m,n]   (psum [128, 2048] in 4 banks)
        h1T = hTpool.tile([128, N], F32, tag="hT")
        ph = [psacc.tile([128, 512], F32, tag="acc", name=f"ph{b}_{i}") for i in range(4)]
        for k in range(NCH):
            for mb in range(4):
                nc.tensor.matmul(
                    ph[mb], Xc[k], T[k][:, 512 * mb : 512 * (mb + 1)],
                    start=(k == 0), stop=(k == NCH - 1),
                )
        for mb in range(4):
            nc.scalar.activation(h1T[:, 512 * mb : 512 * (mb + 1)], ph[mb], AF.Relu)

        # h2[m, hd] = sum_d h1T[d, m] W1[d, hd] + b1
        h2a = hddpool.tile([128, NCH, 128], F32R, tag="hdd")
        for k in range(NCH):
            p2 = pssm.tile([128, 128], F32, tag="pssm")
            nc.tensor.matmul(p2, h1T[:, 128 * k : 128 * (k + 1)], W1s)
            h2f = jkpool.tile([128, 128], F32, tag="junk", name=f"h2f{b}_{k}")
            nc.vector.tensor_add(h2f, p2, b1row)
            dma(out=h2a[:, k, :], in_=h2f.bitcast(F32R))

        # tT[hd, m] = sum_n h2[n, hd] * adj[m, n]
        tT = hTpool.tile([128, N], F32, tag="hT")
        pt = [psacc.tile([128, 512], F32, tag="acc", name=f"pt{b}_{i}") for i in range(4)]
        for k in range(NCH):
            for mb in range(4):
                nc.tensor.matmul(
                    pt[mb], h2a[:, k, :], T[k][:, 512 * mb : 512 * (mb + 1)],
                    start=(k == 0), stop=(k == NCH - 1),
                )
        for mb in range(4):
            nc.scalar.copy(tT[:, 512 * mb : 512 * (mb + 1)], pt[mb])

        # h[m, hd] = (sum_hd tT[hd, m] W2 + b2) * mask
        ha = hddpool.tile([128, NCH, 128], F32, tag="hdd")
        for k in range(NCH):
            p2 = pssm.tile([128, 128], F32, tag="pssm")
            nc.tensor.matmul(p2, tT[:, 128 * k : 128 * (k + 1)], W2s)
            nc.vector.tensor_add(ha[:, k, :], p2, b2row)
            nc.vector.tensor_scalar(
                ha[:, k, :], ha[:, k, :], mask_sb[:, k : k + 1], None, op0=OP.mult
            )

        # logits[m] = h[m,:] . wa  (+ (mask-1)*NEG), layout [128, 16]
        lg = smpool.tile([128, NCH], F32, tag="lg")
        for k in range(NCH):
            junk = jkpool.tile([128, 128], F32, tag="junk")
            nc.vector.tensor_mul(junk, ha[:, k, :], warow)
            nc.vector.tensor_reduce(lg[:, k : k + 1], junk, AX.X, OP.add)
        mshift = smpool.tile([128, NCH], F32, tag="mshift")
        nc.vector.tensor_scalar(mshift, mask_sb, 1.0, NEG, op0=OP.subtract, op1=OP.mult)
        nc.vector.tensor_add(lg, lg, mshift)

        if STAGE < 2:
            continue
        # ================= PHASE B: softmax + top-k selection =================
        lmaxp = smpool.tile([128, 1], F32, tag="lmaxp")
        nc.vector.tensor_reduce(lmaxp, lg, AX.X, OP.max)
        lmax1 = xreduce(lmaxp, OP.max, "lmax1")
        lmaxc = bc(lmax1, "lmaxc")

        d16 = smpool.tile([128, NCH], F32, tag="d16")
        nc.vector.tensor_scalar(d16, lg, lmaxc, None, op0=OP.subtract)
        dcl = smpool.tile([128, NCH], F32, tag="dcl")
        nc.vector.tensor_scalar(dcl, d16, -100.0, None, op0=OP.max)
        e16 = smpool.tile([128, NCH], F32, tag="e16")
        nc.scalar.activation(e16, dcl, AF.Exp)
        guard = smpool.tile([128, NCH], F32, tag="guard")
        nc.vector.tensor_scalar(guard, d16, -95.0, None, op0=OP.is_ge)
        nc.vector.tensor_mul(e16, e16, guard)

        sum1 = xsum_all(e16, "sum1")
        rz1 = smpool.tile([1, 1], F32, tag="rz1")
        nc.vector.reciprocal(rz1, sum1)
        rz = bc(rz1, "rz")
        att16 = smpool.tile([128, NCH], F32, tag="att16")
        nc.vector.tensor_scalar(att16, e16, rz, None, op0=OP.mult)

        # Z = att * h
        Za = zpool.tile([128, NCH, 128], F32R, tag="z")
        for k in range(NCH):
            zf = jkpool.tile([128, 128], F32, tag="junk", name=f"zf{b}_{k}")
            nc.vector.tensor_scalar(
                zf, ha[:, k, :], att16[:, k : k + 1], None, op0=OP.mult
            )
            dma(out=Za[:, k, :], in_=zf.bitcast(F32R))

        # out = h.sum(0) / (EPS + mask.sum())
        po = psrow.tile([1, 128], F32, tag="pssm")
        for k in range(NCH):
            nc.tensor.matmul(po, ones_f32, ha[:, k, :],
                             start=(k == 0), stop=(k == NCH - 1))
        msum1 = xsum_all(mask_sb, "msum1")
        den1 = smpool.tile([1, 1], F32, tag="den1")
        nc.vector.tensor_scalar(den1, msum1, EPS, None, op0=OP.add)
        rden = smpool.tile([1, 1], F32, tag="rden")
        nc.vector.reciprocal(rden, den1)
        outrow = smpool.tile([1, 128], F32, tag="outrow")
        nc.vector.tensor_scalar(outrow, po, rden, None, op0=OP.mult)
        dma(out=out_h[b, :].unsqueeze(0), in_=outrow)

        # valid[r] = r < 0.25*msum  (== r < ceil(0.25*msum) for integer r)
        kq1 = smpool.tile([1, 1], F32, tag="kq1")
        nc.vector.tensor_scalar(kq1, msum1, 0.25, None, op0=OP.mult)
        kqc = bc(kq1, "kqc")
        validrow = perpool.tile([128, K], F32, tag="validrow")
        nc.vector.tensor_scalar(validrow, i512row, kqc, None, op0=OP.is_lt)
        dma(out=nmask_h[b, :].unsqueeze(0), in_=validrow[0:1, :])

        # threshold pass 1: t = 512th largest att
        kth1 = smpool.tile([1, 2], F32, tag="kth1")
        nc.gpsimd.kth_largest(kth1, att16, n_per_lane=NCH, k=510, quantile=QUANT)
        tcol = bc(kth1[0:1, 1:2], "tcol")

        # keys: att if att>t ; -(m+1) if att==t ; -5000 else
        gt16 = smpool.tile([128, NCH], U8, tag="gt16")
        nc.vector.tensor_scalar(gt16, att16, tcol, None, op0=OP.is_gt)
        eq16 = smpool.tile([128, NCH], U8, tag="eq16")
        nc.vector.tensor_scalar(eq16, att16, tcol, None, op0=OP.is_equal)
        key16 = smpool.tile([128, NCH], F32, tag="key16")
        nc.vector.memset(key16, -5000.0)
        nc.vector.copy_predicated(key16, eq16, negidxc)
        nc.vector.copy_predicated(key16, gt16, att16)

        kth2 = smpool.tile([1, 2], F32, tag="kth2")
        nc.gpsimd.kth_largest(kth2, key16, n_per_lane=NCH, k=510, quantile=QUANT)
        t2col = bc(kth2[0:1, 1:2], "t2col")
        sel16 = smpool.tile([128, NCH], U8, tag="sel16")
        nc.vector.tensor_scalar(sel16, key16, t2col, None, op0=OP.is_ge)

        if STAGE < 3:
            continue
        # compaction inputs: value where selected else -1
        attm = smpool.tile([128, NCH], F32, tag="attm")
        nc.vector.memset(attm, -1.0)
        nc.vector.copy_predicated(attm, sel16, att16)
        idxm = smpool.tile([128, NCH], F32, tag="idxm")
        nc.vector.memset(idxm, -1.0)
        nc.vector.copy_predicated(idxm, sel16, iotamc)

        # bounce to DRAM flat (m = t*128 + p), reload in [16,128] sg-wrap (pos = f*16+p)
        dma(out=bass.AP(scr_h, b * 5120 + 0, [[1, 128], [128, NCH]]), in_=attm)
        dma(out=bass.AP(scr_h, b * 5120 + 2048, [[1, 128], [128, NCH]]), in_=idxm)
        att_sg = jkpool.tile([16, 128], F32, tag="junk", name=f"attsg{b}")
        dma(out=att_sg, in_=bass.AP(scr_h, b * 5120 + 0, [[1, 16], [16, 128]]))
        idx_sg = jkpool.tile([16, 128], F32, tag="junk", name=f"idxsg{b}")
        dma(out=idx_sg, in_=bass.AP(scr_h, b * 5120 + 2048, [[1, 16], [16, 128]]))

        attC = smpool.tile([16, 32], F32, tag="attC")
        nf1 = smpool.tile([1, 1], U32, tag="nf1")
        nc.gpsimd.sparse_gather(attC, att_sg, num_found=nf1)
        idxC = smpool.tile([16, 32], F32, tag="idxC")
        nf2 = smpool.tile([1, 1], U32, tag="nf2")
        nc.gpsimd.sparse_gather(idxC, idx_sg, num_found=nf2)

        dma(out=bass.AP(scr_h, b * 5120 + 4096, [[1, 16], [16, 32]]), in_=attC)
        dma(out=bass.AP(scr_h, b * 5120 + 4608, [[1, 16], [16, 32]]), in_=idxC)
        Ratt = perpool.tile([128, K], F32, tag="Ratt")
        dma(out=Ratt, in_=bass.AP(scr_h, b * 5120 + 4096, [[0, 128], [1, K]]))
        Catt = smpool.tile([128, 4], F32, tag="Catt")
        dma(out=Catt, in_=bass.AP(scr_h, b * 5120 + 4096, [[1, 128], [128, 4]]))
        Cidx = smpool.tile([128, 4], F32, tag="Cidx")
        dma(out=Cidx, in_=bass.AP(scr_h, b * 5120 + 4608, [[1, 128], [128, 4]]))

        # rank pass: rank = #(att' > att) + #(att' == att & pos' < pos), then
        # scatter: S[rank] = orig_idx via one-hot matmul
        pS = psrow.tile([1, K], F32, tag="pssm")
        for c in range(4):
            posc = smpool.tile([128, 1], F32, tag="posc")
            nc.vector.tensor_scalar(posc, i128col, float(128 * c), None, op0=OP.add)
            buf1 = selpool.tile([128, K], F32, tag="sel")
            nc.vector.tensor_scalar(buf1, i512row, posc, None, op0=OP.is_lt)
            buf2 = selpool.tile([128, K], F32, tag="sel")
            nc.vector.tensor_scalar(buf2, Ratt, Catt[:, c : c + 1], None, op0=OP.is_equal)
            nc.vector.tensor_mul(buf2, buf2, buf1)
            nc.vector.tensor_scalar(buf1, Ratt, Catt[:, c : c + 1], None, op0=OP.is_gt)
            nc.vector.tensor_add(buf1, buf1, buf2)
            rankc = smpool.tile([128, 1], F32, tag="rankc")
            nc.vector.tensor_reduce(rankc, buf1, AX.X, OP.add)
            ohc = selpool.tile([128, K], F32, tag="sel", name=f"ohc{b}_{c}")
            nc.vector.tensor_scalar(ohc, i512row, rankc, None, op0=OP.is_equal)
            nc.tensor.matmul(pS, Cidx[:, c : c + 1], ohc,
                             start=(c == 0), stop=(c == 3))
        S16 = smpool.tile([1, K], I16, tag="S16")
        nc.vector.tensor_copy(S16, pS)
        dma(out=bass.AP(scrS_h, b * K, [[0, 1], [1, K]]), in_=S16)
        idxw = smpool.tile([128, 32], I16, tag="idxw")
        for g in range(8):
            dma(out=idxw[16 * g : 16 * (g + 1), :],
                in_=bass.AP(scrS_h, b * K, [[1, 16], [16, 32]]))

        if STAGE < 4:
            continue
        # gather A^T columns from T, then normalize:
        # AT[n,k] = valid_k * T[n, S_k] / (sum_k valid_k*T[n,S_k] + EPS)
        AT = atpool.tile([128, NCH, K], F32R, tag="at")
        cs = smpool.tile([128, NCH], F32, tag="cs")
        rcs = smpool.tile([128, NCH], F32, tag="rcs")
        for k in range(NCH):
            atf = selpool.tile([128, K], F32, tag="sel", name=f"atf{b}_{k}")
            nc.gpsimd.ap_gather(atf, f(T[k]), idxw,
                                channels=128, num_elems=N, d=1, num_idxs=K)
            nc.vector.tensor_mul(atf, atf, validrow)
            nc.vector.tensor_reduce(cs[:, k : k + 1], atf, AX.X, OP.add)
            nc.vector.tensor_scalar(cs[:, k : k + 1], cs[:, k : k + 1], EPS, None, op0=OP.add)
            nc.vector.reciprocal(rcs[:, k : k + 1], cs[:, k : k + 1])
            nc.vector.tensor_scalar(
                atf, atf, rcs[:, k : k + 1], None, op0=OP.mult
            )
            dma(out=AT[:, k, :], in_=atf.bitcast(F32R))

        if STAGE < 5:
            continue
        # ================= PHASE C: A-products =================
        # Y[n,j] = sum_m T[m,n] AT[m,j] ; new_adj[k,j] = sum_n AT[n,k] Y[n,j]
        # H_outT[d,k] = sum_n Z[n,d] AT[n,k]
        pna = [psacc.tile([128, K], F32, tag="acc", name=f"pna{b}_{i}") for i in range(4)]
        pho = psacc.tile([128, K], F32, tag="acc")
        for n in range(NCH):
            pY = psy.tile([128, K], F32, tag="psy")
            for m in range(NCH):
                nc.tensor.matmul(
                    pY, T[m][:, 128 * n : 128 * (n + 1)], AT[:, m, :],
                    start=(m == 0), stop=(m == NCH - 1),
                )
            Yf = selpool.tile([128, K], F32, tag="sel", name=f"yf{b}_{n}")
            nc.scalar.copy(Yf, pY)
            Yn = ypool.tile([128, K], F32R, tag="y")
            dma(out=Yn, in_=Yf.bitcast(F32R))
            for kb in range(4):
                nc.tensor.matmul(
                    pna[kb], AT[:, n, 128 * kb : 128 * (kb + 1)], Yn,
                    start=(n == 0), stop=(n == NCH - 1),
                )
            nc.tensor.matmul(pho, Za[:, n, :], AT[:, n, :],
                             start=(n == 0), stop=(n == NCH - 1))

        for kb in range(4):
            stg = selpool.tile([128, K], F32, tag="sel", name=f"stg{b}_{kb}")
            nc.vector.tensor_copy(stg, pna[kb])
            dma(out=nadj_h[b, 128 * kb : 128 * (kb + 1), :], in_=stg)

        hoT = selpool.tile([128, K], F32, tag="sel", name=f"hoT{b}")
        nc.scalar.copy(hoT, pho)
        for kb in range(4):
            ptr = pssm.tile([128, 128], F32, tag="pssm")
            nc.tensor.transpose(ptr, hoT[:, 128 * kb : 128 * (kb + 1)], identsb)
            stg2 = jkpool.tile([128, 128], F32, tag="junk", name=f"stg2_{b}_{kb}")
            nc.vector.tensor_copy(stg2, ptr)
            dma(out=hout_h[b, 128 * kb : 128 * (kb + 1), :], in_=stg2)


_NC_CACHE = {}


def get_nc():
    if "nc" not in _NC_CACHE:
        _NC_CACHE["nc"] = build_nc()
    return _NC_CACHE["nc"]


def make_in_maps(X, adj, mask, W1, b1, W2, b2, w_a):
    consts = {
        "w1": np.ascontiguousarray(W1, np.float32),
        "w2": np.ascontiguousarray(W2, np.float32),
        "b1": np.ascontiguousarray(b1, np.float32),
        "b2": np.ascontiguousarray(b2, np.float32),
        "wa": np.ascontiguousarray(w_a[0, :, 0], np.float32),
        "iota512": np.arange(K, dtype=np.float32),
        "iota128": np.arange(128, dtype=np.float32),
        "iotam": np.arange(N, dtype=np.float32),
        "negidx": -(np.arange(N, dtype=np.float32) + 1.0),
        "ident": np.eye(128, dtype=np.float32),
    }
    in_maps = []
    for c in range(NCORES):
        sl = slice(c * BL, (c + 1) * BL)
        in_maps.append({
            "adjT": np.ascontiguousarray(np.transpose(adj[sl], (0, 2, 1)), np.float32),
            "x": np.ascontiguousarray(X[sl], np.float32),
            "mask": np.ascontiguousarray(mask[sl], np.float32),
            **consts,
        })
    return in_maps


def kernel(X, adj, mask, W1, b1, W2, b2, w_a):
    X, adj, mask = np.asarray(X), np.asarray(adj), np.asarray(mask)
    W1, b1, W2, b2, w_a = map(np.asarray, (W1, b1, W2, b2, w_a))
    nc = get_nc()
    in_maps = make_in_maps(X, adj, mask, W1, b1, W2, b2, w_a)
    res = run_bass_kernel_spmd(nc, in_maps, core_ids=list(range(NCORES)))
    results = res.results
    out = np.concatenate([results[c]["out"] for c in range(NCORES)], axis=0)
    hout = np.concatenate([results[c]["hout"] for c in range(NCORES)], axis=0)
    nadj = np.concatenate([results[c]["nadj"] for c in range(NCORES)], axis=0)
    nmask = np.concatenate([results[c]["nmask"] for c in range(NCORES)], axis=0)
    return (out, hout, nadj, nmask)


# revision 30
# speedup vs baseline: 1.1239x; 1.1239x over previous
"""Trainium2 Bass kernel for nn_AGCNBlock (gnn_message_passing).

Data-parallel over batch: 16 batches -> 8 cores x 2 batches.

Per-batch math (N=2048 nodes, D=HD=128, K=512):
  h   = (adj @ relu(adj@X) @ W1 + b1) @ ... 2-layer GCN -> h [N,HD]
  out = mean-pool(h);  att = softmax(h @ w_a);  Z = att*h
  top-512 nodes by att (desc, ties by index asc — matches jax.lax.top_k)
  A   = row-gathered adj, column-normalized -> new_adj = A adj A^T, H_out = A Z

Layout trick: only adj^T ("T", [col, row]) is kept in SBUF.  Every matmul
(including A-products) consumes T either as stationary or moving, and the
assign matrix is built directly in A^T layout by a free-axis gather of T's
columns (gpsimd ap_gather).  All matmuls run as float32r (full fp32 precision
at 1 cycle/row for moving-free >= 256) because the top-k ordering is
sensitive to ~1e-6 logit noise.
"""

import sys

sys.path.insert(0, "/opt/trn_rl_repo")

import numpy as np

import concourse.bass as bass
import concourse.bacc as bacc
import concourse.mybir as mybir
import concourse.tile as tile
from concourse.bass_utils import run_bass_kernel_spmd

F32 = mybir.dt.float32
F32R = mybir.dt.float32r
I16 = mybir.dt.int16
U32 = mybir.dt.uint32
U8 = mybir.dt.uint8
AF = mybir.ActivationFunctionType
OP = mybir.AluOpType
AX = mybir.AxisListType

import os as _os
STAGE = int(_os.environ.get("KSTAGE", "5"))

B, N, D, HD = 16, 2048, 128, 128
K = 512
NCORES = 8
BL = B // NCORES  # batches per core
NCH = N // 128    # 16 partition-chunks
EPS = 1e-10
NEG = 1e10
QUANT = 1.0 - 510.5 / 2047.0  # kth_largest: k_adj=510 -> out[0,1]=desc[511]


def r(ap):
    return ap.bitcast(F32R)


def f(ap):
    return ap.bitcast(F32)


def build_nc():
    nc = bacc.Bacc(num_swdge_queues=4)

    adjT_h = nc.declare_dram_parameter("adjT", [BL, N, N], F32R, isOutput=False)
    adjn_h = nc.declare_dram_parameter("adjn", [BL, N, N], F32, isOutput=False)
    x_h = nc.declare_dram_parameter("x", [BL, N, D], F32R, isOutput=False)
    mask_h = nc.declare_dram_parameter("mask", [BL, N], F32, isOutput=False)
    w1_h = nc.declare_dram_parameter("w1", [D, HD], F32, isOutput=False)
    w2_h = nc.declare_dram_parameter("w2", [HD, HD], F32, isOutput=False)
    b1_h = nc.declare_dram_parameter("b1", [HD], F32, isOutput=False)
    b2_h = nc.declare_dram_parameter("b2", [HD], F32, isOutput=False)
    wa_h = nc.declare_dram_parameter("wa", [HD], F32, isOutput=False)
    iota512_h = nc.declare_dram_parameter("iota512", [K], F32, isOutput=False)
    iota128_h = nc.declare_dram_parameter("iota128", [128], F32, isOutput=False)
    iotam_h = nc.declare_dram_parameter("iotam", [N], F32, isOutput=False)
    negidx_h = nc.declare_dram_parameter("negidx", [N], F32, isOutput=False)
    ident_h = nc.declare_dram_parameter("ident", [128, 128], F32, isOutput=False)

    out_h = nc.declare_dram_parameter("out", [BL, HD], F32, isOutput=True)
    hout_h = nc.declare_dram_parameter("hout", [BL, K, HD], F32, isOutput=True)
    nadj_h = nc.declare_dram_parameter("nadj", [BL, K, K], F32, isOutput=True)
    nmask_h = nc.declare_dram_parameter("nmask", [BL, K], F32, isOutput=True)

    # DRAM scratch for layout bounces (per batch: attm@0 idxm@2048 attC@4096 idxC@4608)
    scr_h = nc.dram_tensor("scr", [BL, 5120], F32)
    scrS_h = nc.dram_tensor("scrS", [BL, K], I16)

    from contextlib import ExitStack

    with tile.TileContext(nc) as tc, ExitStack() as ctx:
        build_tile(tc, nc, {**locals(), "ctx": ctx})
    nc.compile()
    return nc


def build_tile(tc, nc, h):
    adjT_h, x_h, mask_h = h["adjT_h"], h["x_h"], h["mask_h"]
    adjn_h = h["adjn_h"]
    w1_h, w2_h, b1_h, b2_h, wa_h = h["w1_h"], h["w2_h"], h["b1_h"], h["b2_h"], h["wa_h"]
    iota512_h, iota128_h, iotam_h, negidx_h, ident_h = (
        h["iota512_h"], h["iota128_h"], h["iotam_h"], h["negidx_h"], h["ident_h"],
    )
    out_h, hout_h, nadj_h, nmask_h = h["out_h"], h["hout_h"], h["nadj_h"], h["nmask_h"]
    scr_h, scrS_h = h["scr_h"], h["scrS_h"]

    dma = nc.sync.dma_start        # bulk loads (T, X, consts)
    dmag = nc.gpsimd.dma_start     # selection pipeline + outputs
    dmas = nc.scalar.dma_start     # launder copies
    ctx = h["ctx"]

    cpool = ctx.enter_context(tc.tile_pool(name="consts", bufs=1))
    tpool = ctx.enter_context(tc.tile_pool(name="T", bufs=NCH))
    xpool = ctx.enter_context(tc.tile_pool(name="x", bufs=2))
    hTpool = ctx.enter_context(tc.tile_pool(name="hT", bufs=1))
    hddpool = ctx.enter_context(tc.tile_pool(name="hdd", bufs=1))
    zpool = ctx.enter_context(tc.tile_pool(name="z", bufs=1))
    atpool = ctx.enter_context(tc.tile_pool(name="at", bufs=1))
    ypool = ctx.enter_context(tc.tile_pool(name="y", bufs=1))
    gpool = ctx.enter_context(tc.tile_pool(name="g", bufs=2))
    selpool = ctx.enter_context(tc.tile_pool(name="sel", bufs=2))   # [128,512] scratch
    perpool = ctx.enter_context(tc.tile_pool(name="per", bufs=1))   # per-batch persistents
    smpool = ctx.enter_context(tc.tile_pool(name="small", bufs=1))  # [128,<=16] tiles
    jkpool = ctx.enter_context(tc.tile_pool(name="junk", bufs=2))   # ttr elementwise dump

    # PSUM: 16KB/partition = 8 banks. One shared accumulate pool (tag "acc",
    # 5 slots: phase A uses 4, phase C uses 4+1), Y pipeline 1, small 1, row 1.
    psacc = ctx.enter_context(tc.tile_pool(name="psacc", bufs=5, space="PSUM"))
    psy = ctx.enter_context(tc.tile_pool(name="psy", bufs=1, space="PSUM"))
    pssm = ctx.enter_context(tc.tile_pool(name="pssm", bufs=2, space="PSUM"))
    psrow = pssm

    # ---- constants ----
    W1s = cpool.tile([128, 128], F32, tag="w1")
    dma(out=W1s, in_=w1_h[:, :])
    W2s = cpool.tile([128, 128], F32, tag="w2")
    dma(out=W2s, in_=w2_h[:, :])
    b1row = cpool.tile([128, 128], F32, tag="b1r")
    dma(out=b1row, in_=bass.AP(b1_h, 0, [[0, 128], [1, 128]]))
    b2row = cpool.tile([128, 128], F32, tag="b2r")
    dma(out=b2row, in_=bass.AP(b2_h, 0, [[0, 128], [1, 128]]))
    warow = cpool.tile([128, 128], F32, tag="war")
    dma(out=warow, in_=bass.AP(wa_h, 0, [[0, 128], [1, 128]]))
    i512row = cpool.tile([128, K], F32, tag="i512")
    dma(out=i512row, in_=bass.AP(iota512_h, 0, [[0, 128], [1, K]]))
    i128col = cpool.tile([128, 1], F32, tag="i128")
    dma(out=i128col, in_=bass.AP(iota128_h, 0, [[1, 128], [1, 1]]))
    iotamc = cpool.tile([128, NCH], F32, tag="iotam")
    dma(out=iotamc, in_=bass.AP(iotam_h, 0, [[1, 128], [128, NCH]]))
    negidxc = cpool.tile([128, NCH], F32, tag="negidx")
    dma(out=negidxc, in_=bass.AP(negidx_h, 0, [[1, 128], [128, NCH]]))
    identsb = cpool.tile([128, 128], F32, tag="ident")
    dma(out=identsb, in_=ident_h[:, :])

    ones_row1 = cpool.tile([1, 128], F32, tag="ones1")
    nc.vector.memset(ones_row1, 1.0)
    ones_f32 = cpool.tile([128, 1], F32, tag="onesf")
    nc.vector.memset(ones_f32, 1.0)

    uid = [0]

    def bc(src11, tag):
        # broadcast [1,1] -> [128,1] via PE: psum[i,0] = ones_row1[0,i]*src
        uid[0] += 1
        pb = pssm.tile([128, 1], F32, tag="pssm", name=f"bc{uid[0]}")
        nc.tensor.matmul(pb, ones_row1, src11)
        dst = smpool.tile([128, 1], F32, tag=tag, name=f"bct{uid[0]}")
        nc.vector.tensor_copy(dst, pb)
        return dst

    def xreduce(src, op, tag):
        # cross-partition reduce [128,1] -> [1,1] via PE transpose + DVE
        uid[0] += 1
        pt_ = pssm.tile([1, 128], F32, tag="pssm", name=f"xr{uid[0]}")
        nc.tensor.transpose(pt_, src, identsb)
        dst = smpool.tile([1, 1], F32, tag=tag, name=f"xrt{uid[0]}")
        nc.vector.tensor_reduce(dst, pt_, AX.X, op)
        return dst

    def xsum_all(src, tag):
        # sum over all elements of [128,F] -> [1,1] via ones matmul + DVE
        uid[0] += 1
        F_ = src.shape[-1]
        pr = psrow.tile([1, F_], F32, tag="pssm", name=f"xs{uid[0]}")
        nc.tensor.matmul(pr, ones_f32, src)
        dst = smpool.tile([1, 1], F32, tag=tag, name=f"xst{uid[0]}")
        nc.vector.tensor_reduce(dst, pr, AX.X, OP.add)
        return dst

    for b in range(BL):
        # ================= PHASE A: h-chain (float32r) =================
        mask_sb = smpool.tile([128, NCH], F32, tag="mask")
        dma(out=mask_sb, in_=bass.AP(mask_h, b * N, [[1, 128], [128, NCH]]))

        # interleave X with T so X[k] lands right after T[k] (X is tiny;
        # h1T's chunk-k matmuls need both, so this streams compute behind DMA)
        T, Xc = [], []
        for k in range(NCH):
            Xk = xpool.tile([128, 128], F32R, tag="x")
            dma(out=Xk, in_=x_h[b, 128 * k : 128 * (k + 1), :])
            Xc.append(Xk)
            Tk = tpool.tile([128, N], F32R, tag="T")
            dma(out=Tk, in_=adjT_h[b, 128 * k : 128 * (k + 1), :])
            T.append(Tk)

        # h1T[d, m] = sum_n X[n,d] * adj[m,n]   (psum [128, 2048] in 4 banks)
        h1T = hTpool.tile([128, N], F32, tag="hT")
        ph = [psacc.tile([128, 512], F32, tag="acc", name=f"ph{b}_{i}") for i in range(4)]
        for k in range(NCH):
            for mb in range(4):
                nc.tensor.matmul(
                    ph[mb], Xc[k], T[k][:, 512 * mb : 512 * (mb + 1)],
                    start=(k == 0), stop=(k == NCH - 1),
                )
        for mb in range(4):
            nc.scalar.activation(h1T[:, 512 * mb : 512 * (mb + 1)], ph[mb], AF.Relu)

        # h2[m, hd] = sum_d h1T[d, m] W1[d, hd] + b1
        h2a = hddpool.tile([128, NCH, 128], F32R, tag="hdd")
        for q in range(4):
            h2f4 = selpool.tile([128, 512], F32, tag="sel", name=f"h2f{b}_{q}")
            for j in range(4):
                k = 4 * q + j
                p2 = pssm.tile([128, 128], F32, tag="pssm", name=f"p2_{b}_{k}")
                nc.tensor.matmul(p2, h1T[:, 128 * k : 128 * (k + 1)], W1s)
                nc.vector.tensor_add(h2f4[:, 128 * j : 128 * (j + 1)], p2, b1row)
            dmas(out=h2a[:, 4 * q : 4 * (q + 1), :], in_=h2f4.bitcast(F32R))

        # tT[hd, m] = sum_n h2[n, hd] * adj[m, n]
        tT = hTpool.tile([128, N], F32, tag="hT")
        pt = [psacc.tile([128, 512], F32, tag="acc", name=f"pt{b}_{i}") for i in range(4)]
        for k in range(NCH):
            for mb in range(4):
                nc.tensor.matmul(
                    pt[mb], h2a[:, k, :], T[k][:, 512 * mb : 512 * (mb + 1)],
                    start=(k == 0), stop=(k == NCH - 1),
                )
        for mb in range(4):
            nc.scalar.copy(tT[:, 512 * mb : 512 * (mb + 1)], pt[mb])

        # h[m, hd] = (sum_hd tT[hd, m] W2 + b2) * mask
        ha = hddpool.tile([128, NCH, 128], F32, tag="hdd")
        for k in range(NCH):
            p2 = pssm.tile([128, 128], F32, tag="pssm")
            nc.tensor.matmul(p2, tT[:, 128 * k : 128 * (k + 1)], W2s)
            nc.vector.tensor_add(ha[:, k, :], p2, b2row)
            nc.vector.tensor_scalar(
                ha[:, k, :], ha[:, k, :], mask_sb[:, k : k + 1], None, op0=OP.mult
            )

        # logits[m] = h[m,:] . wa  (+ (mask-1)*NEG), layout [128, 16]
        lg = smpool.tile([128, NCH], F32, tag="lg")
        for k in range(NCH):
            junk = jkpool.tile([128, 128], F32, tag="junk")
            nc.vector.tensor_mul(junk, ha[:, k, :], warow)
            nc.vector.tensor_reduce(lg[:, k : k + 1], junk, AX.X, OP.add)
        mshift = smpool.tile([128, NCH], F32, tag="mshift")
        nc.vector.tensor_scalar(mshift, mask_sb, 1.0, NEG, op0=OP.subtract, op1=OP.mult)
        nc.vector.tensor_add(lg, lg, mshift)

        if STAGE < 2:
            continue
        # ================= PHASE B: softmax + top-k selection =================
        lmaxp = smpool.tile([128, 1], F32, tag="lmaxp")
        nc.vector.tensor_reduce(lmaxp, lg, AX.X, OP.max)
        lmax1 = xreduce(lmaxp, OP.max, "lmax1")
        lmaxc = bc(lmax1, "lmaxc")

        d16 = smpool.tile([128, NCH], F32, tag="d16")
        nc.vector.tensor_scalar(d16, lg, lmaxc, None, op0=OP.subtract)
        dcl = smpool.tile([128, NCH], F32, tag="dcl")
        nc.vector.tensor_scalar(dcl, d16, -100.0, None, op0=OP.max)
        e16 = smpool.tile([128, NCH], F32, tag="e16")
        nc.scalar.activation(e16, dcl, AF.Exp)
        guard = smpool.tile([128, NCH], F32, tag="guard")
        nc.vector.tensor_scalar(guard, d16, -95.0, None, op0=OP.is_ge)
        nc.vector.tensor_mul(e16, e16, guard)

        sum1 = xsum_all(e16, "sum1")
        rz1 = smpool.tile([1, 1], F32, tag="rz1")
        nc.vector.reciprocal(rz1, sum1)
        rz = bc(rz1, "rz")
        att16 = smpool.tile([128, NCH], F32, tag="att16")
        nc.vector.tensor_scalar(att16, e16, rz, None, op0=OP.mult)

        # Z = att * h
        Za = zpool.tile([128, NCH, 128], F32R, tag="z")
        for q in range(4):
            zf4 = selpool.tile([128, 512], F32, tag="sel", name=f"zf{b}_{q}")
            for j in range(4):
                k = 4 * q + j
                nc.vector.tensor_scalar(
                    zf4[:, 128 * j : 128 * (j + 1)], ha[:, k, :],
                    att16[:, k : k + 1], None, op0=OP.mult
                )
            dmas(out=Za[:, 4 * q : 4 * (q + 1), :], in_=zf4.bitcast(F32R))

        # out = h.sum(0) / (EPS + mask.sum())
        po = psrow.tile([1, 128], F32, tag="pssm")
        for k in range(NCH):
            nc.tensor.matmul(po, ones_f32, ha[:, k, :],
                             start=(k == 0), stop=(k == NCH - 1))
        msum1 = xsum_all(mask_sb, "msum1")
        den1 = smpool.tile([1, 1], F32, tag="den1")
        nc.vector.tensor_scalar(den1, msum1, EPS, None, op0=OP.add)
        rden = smpool.tile([1, 1], F32, tag="rden")
        nc.vector.reciprocal(rden, den1)
        outrow = smpool.tile([1, 128], F32, tag="outrow")
        nc.vector.tensor_scalar(outrow, po, rden, None, op0=OP.mult)
        dmag(out=out_h[b, :].unsqueeze(0), in_=outrow)

        # valid[r] = r < 0.25*msum  (== r < ceil(0.25*msum) for integer r)
        kq1 = smpool.tile([1, 1], F32, tag="kq1")
        nc.vector.tensor_scalar(kq1, msum1, 0.25, None, op0=OP.mult)
        kqc = bc(kq1, "kqc")
        validrow = perpool.tile([128, K], F32, tag="validrow")
        nc.vector.tensor_scalar(validrow, i512row, kqc, None, op0=OP.is_lt)
        dmag(out=nmask_h[b, :].unsqueeze(0), in_=validrow[0:1, :])

        # threshold pass 1: t = 512th largest att
        kth1 = smpool.tile([1, 2], F32, tag="kth1")
        nc.gpsimd.kth_largest(kth1, att16, n_per_lane=NCH, k=510, quantile=QUANT)
        tcol = bc(kth1[0:1, 1:2], "tcol")

        # keys: att if att>t ; -(m+1) if att==t ; -5000 else
        gt16 = smpool.tile([128, NCH], U8, tag="gt16")
        nc.vector.tensor_scalar(gt16, att16, tcol, None, op0=OP.is_gt)
        eq16 = smpool.tile([128, NCH], U8, tag="eq16")
        nc.vector.tensor_scalar(eq16, att16, tcol, None, op0=OP.is_equal)
        key16 = smpool.tile([128, NCH], F32, tag="key16")
        nc.vector.memset(key16, -5000.0)
        nc.vector.copy_predicated(key16, eq16, negidxc)
        nc.vector.copy_predicated(key16, gt16, att16)

        kth2 = smpool.tile([1, 2], F32, tag="kth2")
        nc.gpsimd.kth_largest(kth2, key16, n_per_lane=NCH, k=510, quantile=QUANT)
        t2col = bc(kth2[0:1, 1:2], "t2col")
        sel16 = smpool.tile([128, NCH], U8, tag="sel16")
        nc.vector.tensor_scalar(sel16, key16, t2col, None, op0=OP.is_ge)

        if STAGE < 3:
            continue
        # compaction inputs: value where selected else -1
        attm = smpool.tile([128, NCH], F32, tag="attm")
        nc.vector.memset(attm, -1.0)
        nc.vector.copy_predicated(attm, sel16, att16)
        idxm = smpool.tile([128, NCH], F32, tag="idxm")
        nc.vector.memset(idxm, -1.0)
        nc.vector.copy_predicated(idxm, sel16, iotamc)

        # bounce to DRAM flat (m = t*128 + p), reload in [16,128] sg-wrap (pos = f*16+p)
        dmag(out=bass.AP(scr_h, b * 5120 + 0, [[1, 128], [128, NCH]]), in_=attm)
        dmag(out=bass.AP(scr_h, b * 5120 + 2048, [[1, 128], [128, NCH]]), in_=idxm)
        ai_sg = selpool.tile([16, 256], F32, tag="sel", name=f"aisg{b}")
        dmag(out=ai_sg, in_=bass.AP(scr_h, b * 5120 + 0, [[1, 16], [16, 256]]))
        att_sg = ai_sg[:, 0:128]
        idx_sg = ai_sg[:, 128:256]

        attC = smpool.tile([16, 32], F32, tag="attC")
        nf1 = smpool.tile([1, 1], U32, tag="nf1")
        nc.gpsimd.sparse_gather(attC, att_sg, num_found=nf1)
        idxC = smpool.tile([16, 32], F32, tag="idxC")
        nf2 = smpool.tile([1, 1], U32, tag="nf2")
        nc.gpsimd.sparse_gather(idxC, idx_sg, num_found=nf2)

        dmag(out=bass.AP(scr_h, b * 5120 + 4096, [[1, 16], [16, 32]]), in_=attC)
        dmag(out=bass.AP(scr_h, b * 5120 + 4608, [[1, 16], [16, 32]]), in_=idxC)
        Ratt = perpool.tile([128, K], F32, tag="Ratt")
        dmag(out=Ratt, in_=bass.AP(scr_h, b * 5120 + 4096, [[0, 128], [1, K]]))
        Cai = smpool.tile([128, 8], F32, tag="Cai")
        dmag(out=Cai, in_=bass.AP(scr_h, b * 5120 + 4096, [[1, 128], [128, 8]]))
        Catt = Cai[:, 0:4]
        Cidx = Cai[:, 4:8]

        # rank pass: rank = #(att' > att) + #(att' == att & pos' < pos), then
        # scatter: S[rank] = orig_idx via one-hot matmul
        pS = psrow.tile([1, K], F32, tag="pssm")
        for c in range(4):
            posc = smpool.tile([128, 1], F32, tag="posc")
            nc.vector.tensor_scalar(posc, i128col, float(128 * c), None, op0=OP.add)
            buf1 = selpool.tile([128, K], F32, tag="sel")
            nc.vector.tensor_scalar(buf1, i512row, posc, None, op0=OP.is_lt)
            buf2 = selpool.tile([128, K], F32, tag="sel")
            nc.vector.tensor_scalar(buf2, Ratt, Catt[:, c : c + 1], None, op0=OP.is_equal)
            nc.vector.tensor_mul(buf2, buf2, buf1)
            nc.vector.tensor_scalar(buf1, Ratt, Catt[:, c : c + 1], None, op0=OP.is_gt)
            nc.vector.tensor_add(buf1, buf1, buf2)
            rankc = smpool.tile([128, 1], F32, tag="rankc")
            nc.vector.tensor_reduce(rankc, buf1, AX.X, OP.add)
            ohc = selpool.tile([128, K], F32, tag="sel", name=f"ohc{b}_{c}")
            nc.vector.tensor_scalar(ohc, i512row, rankc, None, op0=OP.is_equal)
            nc.tensor.matmul(pS, Cidx[:, c : c + 1], ohc,
                             start=(c == 0), stop=(c == 3))
        S16 = smpool.tile([1, K], I16, tag="S16")
        nc.vector.tensor_copy(S16, pS)
        dmag(out=bass.AP(scrS_h, b * K, [[0, 1], [1, K]]), in_=S16)
        idxw = smpool.tile([128, 32], I16, tag="idxw")
        for g in range(8):
            dmag(out=idxw[16 * g : 16 * (g + 1), :],
                in_=bass.AP(scrS_h, b * K, [[1, 16], [16, 32]]))

        if STAGE < 4:
            continue
        # gather A^T columns from T, then normalize:
        # AT[n,k] = valid_k * T[n, S_k] / (sum_k valid_k*T[n,S_k] + EPS)
        # dma_gather rows of natural adj from DRAM in 256-col slices,
        # PE-transpose into A^T layout, fold valid + column-normalize, launder.
        AT = atpool.tile([128, NCH, K], F32R, tag="at")
        cs = smpool.tile([128, NCH], F32, tag="cs")
        rcs = smpool.tile([128, NCH], F32, tag="rcs")
        for c2 in range(8):  # 256 n-columns per slice
            atfs = [selpool.tile([128, K], F32, tag="sel", name=f"atf{b}_{c2}_{j}")
                    for j in range(2)]
            for kb in range(4):
                Gt = gpool.tile([128, 256], F32, tag="g", name=f"gt{b}_{c2}_{kb}")
                nc.gpsimd.dma_gather(
                    out_ap=Gt.unsqueeze(1),
                    in_ap=bass.AP(adjn_h, b * N * N + 256 * c2, [[N, N], [1, 256]]),
                    idxs_ap=idxw[:, 8 * kb : 8 * (kb + 1)],
                    num_idxs=128, num_idxs_reg=128,
                    elem_size=256, elem_step=N,
                    queue_num=0,
                )
                for j2 in range(2):
                    ptr = pssm.tile([128, 128], F32, tag="pssm", name=f"gtr{b}_{c2}_{kb}_{j2}")
                    nc.tensor.transpose(ptr, Gt[:, 128 * j2 : 128 * (j2 + 1)], identsb)
                    nc.vector.tensor_mul(atfs[j2][:, 128 * kb : 128 * (kb + 1)], ptr,
                                         validrow[:, 128 * kb : 128 * (kb + 1)])
            for j2 in range(2):
                nch = 2 * c2 + j2
                atf = atfs[j2]
                nc.vector.tensor_reduce(cs[:, nch : nch + 1], atf, AX.X, OP.add)
                nc.vector.tensor_scalar(cs[:, nch : nch + 1], cs[:, nch : nch + 1],
                                        EPS, None, op0=OP.add)
                nc.vector.reciprocal(rcs[:, nch : nch + 1], cs[:, nch : nch + 1])
                nc.vector.tensor_scalar(atf, atf, rcs[:, nch : nch + 1], None, op0=OP.mult)
                dmag(out=AT[:, nch, :], in_=atf.bitcast(F32R))

        if STAGE < 5:
            continue
        # ================= PHASE C: A-products =================
        # Y[n,j] = sum_m T[m,n] AT[m,j] ; new_adj[k,j] = sum_n AT[n,k] Y[n,j]
        # H_outT[d,k] = sum_n Z[n,d] AT[n,k]
        pna = [psacc.tile([128, K], F32, tag="acc", name=f"pna{b}_{i}") for i in range(4)]
        pho = psacc.tile([128, K], F32, tag="acc")
        for n in range(NCH):
            pY = psy.tile([128, K], F32, tag="psy")
            for m in range(NCH):
                nc.tensor.matmul(
                    pY, T[m][:, 128 * n : 128 * (n + 1)], AT[:, m, :],
                    start=(m == 0), stop=(m == NCH - 1),
                )
            Yf = selpool.tile([128, K], F32, tag="sel", name=f"yf{b}_{n}")
            nc.scalar.copy(Yf, pY)
            Yn = ypool.tile([128, K], F32R, tag="y")
            dmas(out=Yn, in_=Yf.bitcast(F32R))
            for kb in range(4):
                nc.tensor.matmul(
                    pna[kb], AT[:, n, 128 * kb : 128 * (kb + 1)], Yn,
                    start=(n == 0), stop=(n == NCH - 1),
                )
            nc.tensor.matmul(pho, Za[:, n, :], AT[:, n, :],
                             start=(n == 0), stop=(n == NCH - 1))

        for kb in range(4):
            stg = selpool.tile([128, K], F32, tag="sel", name=f"stg{b}_{kb}")
            nc.vector.tensor_copy(stg, pna[kb])
            dmag(out=nadj_h[b, 128 * kb : 128 * (kb + 1), :], in_=stg)

        hoT = selpool.tile([128, K], F32, tag="sel", name=f"hoT{b}")
        nc.scalar.copy(hoT, pho)
        for kb in range(4):
            ptr = pssm.tile([128, 128], F32, tag="pssm")
            nc.tensor.transpose(ptr, hoT[:, 128 * kb : 128 * (kb + 1)], identsb)
            stg2 = jkpool.tile([128, 128], F32, tag="junk", name=f"stg2_{b}_{kb}")
            nc.vector.tensor_copy(stg2, ptr)
            dmag(out=hout_h[b, 128 * kb : 128 * (kb + 1), :], in_=stg2)


_NC_CACHE = {}


def get_nc():
    if "nc" not in _NC_CACHE:
        _NC_CACHE["nc"] = build_nc()
    return _NC_CACHE["nc"]


def make_in_maps(X, adj, mask, W1, b1, W2, b2, w_a):
    consts = {
        "w1": np.ascontiguousarray(W1, np.float32),
        "w2": np.ascontiguousarray(W2, np.float32),
        "b1": np.ascontiguousarray(b1, np.float32),
        "b2": np.ascontiguousarray(b2, np.float32),
        "wa": np.ascontiguousarray(w_a[0, :, 0], np.float32),
        "iota512": np.arange(K, dtype=np.float32),
        "iota128": np.arange(128, dtype=np.float32),
        "iotam": np.arange(N, dtype=np.float32),
        "negidx": -(np.arange(N, dtype=np.float32) + 1.0),
        "ident": np.eye(128, dtype=np.float32),
    }
    in_maps = []
    for c in range(NCORES):
        sl = slice(c * BL, (c + 1) * BL)
        in_maps.append({
            "adjT": np.ascontiguousarray(np.transpose(adj[sl], (0, 2, 1)), np.float32),
            "adjn": np.ascontiguousarray(adj[sl], np.float32),
            "x": np.ascontiguousarray(X[sl], np.float32),
            "mask": np.ascontiguousarray(mask[sl], np.float32),
            **consts,
        })
    return in_maps


def kernel(X, adj, mask, W1, b1, W2, b2, w_a):
    X, adj, mask = np.asarray(X), np.asarray(adj), np.asarray(mask)
    W1, b1, W2, b2, w_a = map(np.asarray, (W1, b1, W2, b2, w_a))
    nc = get_nc()
    in_maps = make_in_maps(X, adj, mask, W1, b1, W2, b2, w_a)
    res = run_bass_kernel_spmd(nc, in_maps, core_ids=list(range(NCORES)))
    results = res.results
    out = np.concatenate([results[c]["out"] for c in range(NCORES)], axis=0)
    hout = np.concatenate([results[c]["hout"] for c in range(NCORES)], axis=0)
    nadj = np.concatenate([results[c]["nadj"] for c in range(NCORES)], axis=0)
    nmask = np.concatenate([results[c]["nmask"] for c in range(NCORES)], axis=0)
    return (out, hout, nadj, nmask)
